# revision 1
# baseline (speedup 1.0000x reference)
"""DiagLinear kernel for 8 TRN2 NeuronCores.

Computes y = x * weight + bias  (weight/bias broadcast over the batch dim).

Strategy: transpose x on the host to xT [IN_SIZE, BATCH] and shard xT's rows
(the in_size dim) across the 8 cores. With in_size on the SBUF partition
axis, weight/bias become per-partition scalars, so the whole elementwise
computation is a single fused DVE tensor_scalar op per tile:
    out = (x * w) + b          (fp32, 2x perf mode)
which keeps the kernel firmly DMA-bound. Measured steady-state DMA rate per
core is ~430-440 GB/s (two concurrent sequential streams, near the 435 GB/s
SBUF-AXI fabric ceiling); 2 x 16.78 MB of traffic per core gives ~82 us of
bus time + ~9 us fixed preamble/postamble.

Each row of the per-core input is augmented on the host with 16 leading
columns (w, b, 14 pad — 64 B total so every DMA descriptor line stays
64B-aligned; 8B-aligned lines measured ~20% slower). Every SBUF tile is
self-contained: the fused op reads its per-partition scalars from columns
0/1 of the tile it just loaded. The kernel is raw Bass (no Tile) with a
fully static schedule: 4 tiles of [128, 16+8192] per core, loads and stores
split across the two HWDGE rings (SP and ACT sequencers) so exactly two
large sequential transfers are in flight at all times (more concurrent
streams measurably degrade HBM efficiency), DVE compute chained behind each
load via standalone semaphore waits.
"""

import numpy as np

import concourse.bass as bass
import concourse.mybir as mybir
from concourse.bass_utils import run_bass_kernel_spmd

N_CORES = 8
IN_SIZE = 4096
BATCH = 8192
P = 128                                # SBUF partitions
ROWS_PER_CORE = IN_SIZE // N_CORES     # 512 rows of xT per core
N_PBLK = ROWS_PER_CORE // P            # 4 partition blocks per core
AUG = 16                               # leading [w, b, pad...] columns per row
                                       # (16 cols = 64 B keeps every DMA line
                                       # 64B-aligned)
W = AUG + BATCH                        # augmented row width

# test.py hooks: set TRACE=True before calling kernel() to capture an NTFF
# profile; the BassKernelResults land in LAST_RESULTS.
TRACE = False
LAST_RESULTS = None

_cached_nc = None


def _build():
    f32 = mybir.dt.float32
    nc = bass.Bass(
        trn_type="TRN2", enable_partition_id=False, monotonic_sem_count=0
    )
    xt = nc.dram_tensor("xt", [ROWS_PER_CORE, W], f32, kind="ExternalInput")
    yt = nc.dram_tensor("yt", [ROWS_PER_CORE, BATCH], f32, kind="ExternalOutput")

    with (
        nc.sbuf_tensor("t0", [P, W], f32) as t0,
        nc.sbuf_tensor("t1", [P, W], f32) as t1,
        nc.sbuf_tensor("t2", [P, W], f32) as t2,
        nc.sbuf_tensor("t3", [P, W], f32) as t3,
        nc.semaphore("in_sp") as in_sp,
        nc.semaphore("in_act") as in_act,
        nc.semaphore("dve_done") as dve_done,
        nc.semaphore("out_sp") as out_sp,
        nc.semaphore("out_act") as out_act,
        nc.Block() as block,
    ):
        tiles = [t0, t1, t2, t3]
        rows = [slice(k * P, (k + 1) * P) for k in range(N_PBLK)]

        # Tiles 0, 2 move on the SP ring; tiles 1, 3 on the ACT ring.
        @block.sync
        def _(sync):
            sync.dma_start(t0[:], xt[rows[0], :]).then_inc(in_sp, 16)
            sync.dma_start(t2[:], xt[rows[2], :]).then_inc(in_sp, 16)
            sync.wait_ge(dve_done, 1)
            sync.dma_start(yt[rows[0], :], t0[:, AUG:]).then_inc(out_sp, 16)
            sync.wait_ge(dve_done, 3)
            sync.dma_start(yt[rows[2], :], t2[:, AUG:]).then_inc(out_sp, 16)
            sync.wait_ge(out_sp, 32)

        @block.scalar
        def _(scalar):
            scalar.dma_start(t1[:], xt[rows[1], :]).then_inc(in_act, 16)
            scalar.dma_start(t3[:], xt[rows[3], :]).then_inc(in_act, 16)
            scalar.wait_ge(dve_done, 2)
            scalar.dma_start(yt[rows[1], :], t1[:, AUG:]).then_inc(out_act, 16)
            scalar.wait_ge(dve_done, 4)
            scalar.dma_start(yt[rows[3], :], t3[:, AUG:]).then_inc(out_act, 16)
            scalar.wait_ge(out_act, 32)

        @block.vector
        def _(vector):
            waits = [(in_sp, 16), (in_act, 16), (in_sp, 32), (in_act, 32)]
            for k, t in enumerate(tiles):
                sem, val = waits[k]
                vector.wait_ge(sem, val)
                vector.tensor_scalar(
                    out=t[:, AUG:],
                    in0=t[:, AUG:],
                    scalar1=t[:, 0:1],
                    scalar2=t[:, 1:2],
                    op0=mybir.AluOpType.mult,
                    op1=mybir.AluOpType.add,
                ).then_inc(dve_done, 1)

    return nc


def kernel(x, weight, bias):
    global LAST_RESULTS, _cached_nc
    x = np.ascontiguousarray(np.asarray(x), dtype=np.float32)
    weight = np.ascontiguousarray(np.asarray(weight), dtype=np.float32)
    bias = np.ascontiguousarray(np.asarray(bias), dtype=np.float32)
    assert x.shape == (BATCH, IN_SIZE)

    # Build the augmented transposed input: row r of xta is
    # [weight[r], bias[r], 0 x 14, x[0, r], x[1, r], ..., x[BATCH-1, r]].
    xta = np.empty((IN_SIZE, W), dtype=np.float32)
    xta[:, 0] = weight
    xta[:, 1] = bias
    xta[:, 2:AUG] = 0.0
    xta[:, AUG:] = x.T

    if _cached_nc is None:
        _cached_nc = _build()
    nc = _cached_nc

    in_maps = []
    for c in range(N_CORES):
        r0 = c * ROWS_PER_CORE
        in_maps.append({"xt": xta[r0:r0 + ROWS_PER_CORE]})

    res = run_bass_kernel_spmd(
        nc, in_maps, core_ids=list(range(N_CORES)), trace=TRACE
    )
    LAST_RESULTS = res
    yT = np.concatenate([r["yt"] for r in res.results], axis=0)  # [IN_SIZE, BATCH]
    return np.ascontiguousarray(yT.T)

